# revision 13
# baseline (speedup 1.0000x reference)
"""Additive (Bahdanau) attention on 8 TRN2 NeuronCores.

scores[b,i,j] = sum_h wv_h * tanh(qp[b,i,h] + kp[b,j,h]),  qp = q@Wq.T, kp = k@Wk.T
masked softmax over j, then attn @ values.

Math: tanh(s) ~ c0*s + sum_n b_n sin(w_n s) with frequencies from two power-of-2
ladders; sin(w(q+k)) = sin(wq)cos(wk)+cos(wq)sin(wk) turns the (B,NQ,NK,H) tanh
contraction into TensorEngine matmuls over Fourier features. ACT Sin is only
accurate for |arg|<=3.15, so base harmonics use Sin directly and higher ones are
built by double-angle ladders. The ladder keeps sine products RAW (sp_n =
sin(n w x)/n') and only exactifies cosines (needed by the chain anyway); all
2^k scales fold into the A-side prescales, with rank-1 alpha/beta correction
rows for the leaf harmonics. Softmax skips the max-subtraction entirely
(scores are bounded ~|4|; masked cols carry -1e6 so exp underflows to 0 like
the reference).

Sharding: keys are sharded across cores. Each core gets (batch b, key-chunk
range) with a common per-core KPAD = 128*L chosen so the ceil(vl_b/128) chunks
of all batches bin-pack into 8 single-batch bins; every core computes partial
ov[b] = E@V and z[b] = sum(E) over its key range for ALL 128 queries of its
batch, and the host combines: out = sum(ov) / sum(z). No cross-core
communication.
"""
import sys
import numpy as np

try:
    import concourse.bass as bass
except ImportError:
    sys.path.insert(0, "/opt/trn_rl_repo")
    import concourse.bass as bass
import concourse.bacc as bacc
import concourse.mybir as mybir
from contextlib import ExitStack
from concourse.tile import TileContext
from concourse.bass_utils import run_bass_kernel_spmd

F32 = mybir.dt.float32
BF = mybir.dt.bfloat16
AF = mybir.ActivationFunctionType
ALU = mybir.AluOpType

B, NQ, NK, H, DV = 4, 128, 1024, 256, 256
PIHALF = float(np.pi / 2)

# tanh(x) ~ C0*x + sum b_(li,n) sin(n * w_li * x); weighted LSQ fit over N(0,1.67^2)
CFG = ((0.44, (1, 2, 4, 8)), (0.32, (4, 8)))
C0 = 0.150435
COEF = {(0, 1): 0.541169, (0, 2): 0.257046, (0, 4): 0.085767,
        (0, 8): 0.008478, (1, 4): 0.089182, (1, 8): 0.031948}
# (ladder, A-feature, K-feature, scale-fold multiplier)
PAIRS = [(0, "s1", "c1", 1.0), (0, "c1", "s1", 1.0),
         (0, "sp2", "c2", 2.0), (0, "c2", "sp2", 2.0),
         (0, "sp4", "c4", 4.0), (0, "c4", "sp4", 4.0),
         (0, "sp8", "ct8", 16.0), (0, "ct8", "sp8", 16.0),
         (1, "sp4", "c4", 4.0), (1, "c4", "sp4", 4.0),
         (1, "sp8", "ct8", 16.0), (1, "ct8", "sp8", 16.0)]


def _pair_n(aname):
    return 1 if aname in ("s1", "c1") else int(aname[-1])


def _chain(nc, pool, src_ap, width, tag, pihalf, ct_gpsimd_li=None, view=None):
    """sin/cos ladder over an fp32 source [128, width].

    Chain per ladder: s1,c1 (ACT Sin) -> sp2=s1*c1, ct2=c1^2, c2=2ct2-1 ->
    sp4=sp2*c2, ct4=c2^2, c4=2ct4-1 -> sp8=sp4*c4, ct8=c4^2.  sp_n = sin(nwx)/n'
    raw; c_n exact.  Returns per-ladder feature dicts (bf16 tiles).
    """
    v = view if view is not None else (lambda ap: ap)
    absx = pool.tile([128, width], F32, name=f"ab{tag}", tag=f"ab{tag}")
    nc.scalar.activation(v(absx[:]), src_ap, AF.Abs)
    feats = []
    for li, (w, _) in enumerate(CFG):
        f = {}
        s1 = pool.tile([128, width], BF, name=f"s1{tag}{li}", tag=f"s1{tag}{li}")
        c1 = pool.tile([128, width], BF, name=f"c1{tag}{li}", tag=f"c1{tag}{li}")
        nc.scalar.activation(v(s1[:]), src_ap, AF.Sin, scale=float(w))
        nc.scalar.activation(c1[:], absx[:], AF.Sin, scale=float(-w),
                             bias=pihalf[:, 0:1])
        f["s1"], f["c1"] = s1, c1
        ps, pc = s1, c1
        n = 1
        while n < 8:
            n *= 2
            sp = pool.tile([128, width], BF, name=f"sp{n}{tag}{li}",
                           tag=f"sp{n}{tag}{li}")
            ct = pool.tile([128, width], BF, name=f"ct{n}{tag}{li}",
                           tag=f"ct{n}{tag}{li}")
            nc.vector.tensor_tensor(sp[:], ps[:], pc[:], ALU.mult)
            eng = nc.gpsimd if ct_gpsimd_li == -1 else nc.vector
            eng.tensor_tensor(ct[:], pc[:], pc[:], ALU.mult)
            f[f"sp{n}"], f[f"ct{n}"] = sp, ct
            if n < 8:
                c = pool.tile([128, width], BF, name=f"c{n}{tag}{li}",
                              tag=f"c{n}{tag}{li}")
                nc.vector.tensor_scalar(c[:], ct[:], 2.0, -1.0, ALU.mult, ALU.add)
                f[f"c{n}"] = c
                ps, pc = sp, c
        feats.append(f)
    return feats


def build_program(KPAD):
    import ml_dtypes
    KC = KPAD // 128
    W2 = 2 * KPAD  # packed k-feature width [hc0 | hc1]

    nc = bacc.Bacc("TRN2", target_bir_lowering=False, debug=False, num_devices=8)
    d_q = nc.declare_dram_parameter("queries", [NQ, H], F32, isOutput=False)
    d_k = nc.declare_dram_parameter("keys", [KPAD, H], F32, isOutput=False)
    d_v = nc.declare_dram_parameter("values", [KPAD, DV], F32, isOutput=False)
    d_wq = nc.declare_dram_parameter("Wq", [H, H], F32, isOutput=False)
    d_wk = nc.declare_dram_parameter("Wk", [H, H], F32, isOutput=False)
    d_wv = nc.declare_dram_parameter("wv", [H, 1], F32, isOutput=False)
    d_vl = nc.declare_dram_parameter("vl", [1, 1], F32, isOutput=False)
    d_uq = nc.declare_dram_parameter("uq", [128, 2], F32, isOutput=False)
    d_uk = nc.declare_dram_parameter("uk", [128, 2], F32, isOutput=False)
    d_ov = nc.declare_dram_parameter("ov", [NQ, DV], F32, isOutput=True)
    d_z = nc.declare_dram_parameter("z", [NQ, 1], F32, isOutput=True)
    d_ident = nc.inline_tensor(np.eye(128).astype(ml_dtypes.bfloat16), name="identbf")
    g = np.arange(KPAD)
    d_iota = nc.inline_tensor(((g % 128) * KC + g // 128).astype(np.float32)
                              .reshape(1, KPAD), name="iotaf")

    with TileContext(nc) as tc, ExitStack() as ex:
        cpool = ex.enter_context(tc.tile_pool(name="consts", bufs=1))
        qpool = ex.enter_context(tc.tile_pool(name="qlad", bufs=1))
        lpool = ex.enter_context(tc.tile_pool(name="klad", bufs=1))
        apool = ex.enter_context(tc.tile_pool(name="aside", bufs=1))
        wpool = ex.enter_context(tc.tile_pool(name="work", bufs=1))
        kpool = ex.enter_context(tc.tile_pool(name="keysin", bufs=1))
        ptp = ex.enter_context(tc.tile_pool(name="ptp", bufs=1, space="PSUM"))
        pprj = ex.enter_context(tc.tile_pool(name="pprj", bufs=1, space="PSUM"))
        pacc = ex.enter_context(tc.tile_pool(name="pacc", bufs=1, space="PSUM"))
        pbp = ex.enter_context(tc.tile_pool(name="pbp", bufs=1, space="PSUM"))
        pq = ex.enter_context(tc.tile_pool(name="pq", bufs=1, space="PSUM"))
        pov = ex.enter_context(tc.tile_pool(name="pov", bufs=1, space="PSUM"))
        psc = ex.enter_context(tc.tile_pool(name="psc", bufs=1, space="PSUM"))

        # ---------------- DMAs (consolidated; values last) ----------------
        ident = cpool.tile([128, 128], BF, name="ident", tag="ident")
        nc.sync.dma_start(ident[:], d_ident[:])
        wq_sb = cpool.tile([128, 512], F32, name="wqsb", tag="wqsb")
        wk_sb = cpool.tile([128, 512], F32, name="wksb", tag="wksb")
        wv_sb = cpool.tile([128, 2], F32, name="wvsb", tag="wvsb")
        vl_sb = cpool.tile([1, 1], F32, name="vl", tag="vl")
        q_sb = cpool.tile([NQ, H], F32, name="qsb", tag="qsb")
        kin = kpool.tile([128, KC * H], F32, name="kin", tag="kin")
        iota_sb = apool.tile([1, KPAD], F32, name="iota", tag="iota")
        vin = kpool.tile([128, KC * DV], F32, name="vin", tag="vin")
        uq_f = cpool.tile([128, 2], F32, name="uqf", tag="uqf")
        uk_f = cpool.tile([128, 2], F32, name="ukf", tag="ukf")
        uq_sb = cpool.tile([128, 2], BF, name="uqsb", tag="uqsb")
        uk_sb = cpool.tile([128, 2], BF, name="uksb", tag="uksb")
        nc.sync.dma_start(kin[:], d_k.rearrange("(p kc) d -> p (kc d)", p=128))
        nc.sync.dma_start(q_sb[:], d_q[:])
        nc.sync.dma_start(wq_sb[:], d_wq.rearrange("(p a) d -> p (a d)", p=128))
        nc.scalar.dma_start(wk_sb[:], d_wk.rearrange("(p a) d -> p (a d)", p=128))
        nc.scalar.dma_start(wv_sb[:], d_wv.rearrange("(p a) o -> p (a o)", p=128))
        nc.scalar.dma_start(uq_f[:], d_uq[:])
        nc.scalar.dma_start(uk_f[:], d_uk[:])
        nc.scalar.dma_start(vl_sb[:], d_vl[:])
        nc.scalar.dma_start(iota_sb[:], d_iota[:])
        nc.scalar.dma_start(vin[:], d_v.rearrange("(p kc) d -> p (kc d)", p=128))

        # PE warmup: dummy accumulating matmuls warm the HAM clock gate
        ov_ps = pov.tile([NQ, DV], F32, name="ov", tag="ov")
        for wi in range(12):
            nc.tensor.matmul(ov_ps[:, 0:128], ident[:], ident[:],
                             start=(wi == 0), stop=False, skip_group_check=True)

        pihalf = cpool.tile([128, 1], F32, name="pihalf", tag="pihalf")
        nc.vector.memset(pihalf[:], PIHALF)
        neg_m05 = cpool.tile([128, 1], BF, name="negm05", tag="negm05")
        nc.vector.memset(neg_m05[:], -0.5)

        # ---------------- casts ----------------
        wq_bf = cpool.tile([128, 512], BF, name="wqbf", tag="wqbf")
        wk_bf = cpool.tile([128, 512], BF, name="wkbf", tag="wkbf")
        q_bf = cpool.tile([NQ, H], BF, name="qbf", tag="qbf")
        nc.vector.tensor_copy(wq_bf[:], wq_sb[:])
        nc.vector.tensor_copy(wk_bf[:], wk_sb[:])
        nc.vector.tensor_copy(q_bf[:], q_sb[:])

        # ---------------- transposes: Wq, queries (PE), then Wk ----------------
        wqT = [cpool.tile([128, 256], BF, name=f"wqT{i}", tag=f"wqT{i}") for i in range(2)]
        wkT = [cpool.tile([128, 256], BF, name=f"wkT{i}", tag=f"wkT{i}") for i in range(2)]
        for a in range(2):
            for dc in range(2):
                ps = ptp.tile([128, 128], BF, name="tp", tag="tp")
                nc.tensor.transpose(ps[:], wq_bf[:, a * 256 + dc * 128:a * 256 + (dc + 1) * 128], ident[:])
                (nc.vector.tensor_copy if a == 0 else nc.scalar.copy)(
                    wqT[dc][:, a * 128:(a + 1) * 128], ps[:])
        qT = [cpool.tile([128, NQ], BF, name=f"qT{i}", tag=f"qT{i}") for i in range(2)]
        for dc in range(2):
            ps = ptp.tile([128, 128], BF, name="tp", tag="tp")
            nc.tensor.transpose(ps[:], q_bf[:, dc * 128:(dc + 1) * 128], ident[:])
            nc.scalar.copy(qT[dc][:], ps[:])

        qprj = pq.tile([128, 256], F32, name="qprj", tag="qprj")[:]
        for a in range(2):
            for dc in range(2):
                nc.tensor.matmul(qprj[:, a * 128:(a + 1) * 128],
                                 wqT[dc][:, a * 128:(a + 1) * 128], qT[dc][:],
                                 start=(dc == 0), stop=(dc == 1))

        for a in range(2):
            for dc in range(2):
                ps = ptp.tile([128, 128], BF, name="tp", tag="tp")
                nc.tensor.transpose(ps[:], wk_bf[:, a * 256 + dc * 128:a * 256 + (dc + 1) * 128], ident[:])
                (nc.vector.tensor_copy if a == 0 else nc.scalar.copy)(
                    wkT[dc][:, a * 128:(a + 1) * 128], ps[:])

        # keys: cast + transpose into kTb [d, dc*KPAD + j]
        k_bf = kpool.tile([128, KC * H], BF, name="kbf", tag="kbf")
        nc.scalar.copy(k_bf[:], kin[:])
        kTb = kpool.tile([128, 2 * KPAD], BF, name="kTb", tag="kTb")
        for jc in range(KC):
            for dc in range(2):
                ps = ptp.tile([128, 128], BF, name="tp", tag="tp")
                nc.tensor.transpose(ps[:], k_bf[:, jc * 256 + dc * 128:jc * 256 + (dc + 1) * 128], ident[:])
                nc.scalar.copy(kTb[:, dc * KPAD + jc * 128:dc * KPAD + (jc + 1) * 128], ps[:])

        # kprj: [h-in-a, a*KPAD + j]
        kprj = pprj.tile([128, 1024], F32, name="kprj", tag="kprj")
        for a in range(2):
            for dc in range(2):
                nc.tensor.matmul(kprj[:, a * 512:a * 512 + KPAD],
                                 wkT[dc][:, a * 128:(a + 1) * 128],
                                 kTb[:, dc * KPAD:(dc + 1) * KPAD],
                                 start=(dc == 0), stop=(dc == 1))

        acc = pacc.tile([128, 136], F32, name="acc", tag="acc")
        nc.vector.tensor_copy(uq_sb[:], uq_f[:])
        nc.vector.tensor_copy(uk_sb[:], uk_f[:])
        u_q_bf = [uq_sb[:, dc:dc + 1] for dc in range(2)]
        u_k_bf = [uk_sb[:, dc:dc + 1] for dc in range(2)]

        # mask row: m01 = (iota >= vl) * -1e6
        m01 = apool.tile([1, KPAD], F32, name="m01", tag="m01")
        nc.gpsimd.tensor_scalar(m01[:], iota_sb[:], vl_sb[0:1, 0:1], -1e6,
                                ALU.is_ge, ALU.mult)

        # ---------------- q-side ladder + prescales ----------------
        qf = _chain(nc, qpool, qprj, 256, "q", pihalf, ct_gpsimd_li=-1)
        # u8[li] = -8*b_(li,8)*wv  (beta leaf correction vectors)
        u8 = [cpool.tile([128, 2], BF, name=f"u8{li}", tag=f"u8{li}") for li in range(2)]
        for li in range(2):
            nc.gpsimd.tensor_scalar(u8[li][:], wv_sb[:, 0:2],
                                     float(-8.0 * COEF[(li, 8)]), None, ALU.mult)
        # af[(li, aname, hc)] = m*coef * wv (x) A-feature
        af = {}
        for (li, aname, kname, m) in PAIRS:
            coef = float(m * COEF[(li, _pair_n(aname))])
            for hc in range(2):
                t = apool.tile([128, NQ], BF, name=f"af{li}{aname}{hc}",
                               tag=f"af{li}{aname}{hc}")
                eng = nc.vector if hc == 0 else nc.gpsimd
                eng.tensor_scalar(t[:], qf[li][aname][:, hc * 128:(hc + 1) * 128],
                                  wv_sb[:, hc:hc + 1], coef, ALU.mult, ALU.mult)
                af[(li, aname, hc)] = t

        # ---------------- k-side ladder ----------------
        kprj_src = kprj[:].rearrange("p (a j) -> p a j", a=2)[:, :, 0:KPAD]
        kview = (lambda ap: ap.rearrange("p (a j) -> p a j", a=2))
        kf = _chain(nc, lpool, kprj_src, W2, "k", pihalf, view=kview)

        # preload the Exp ACT table off the critical path (dep on last Sin)
        escr = wpool.tile([1, 1], F32, name="escr", tag="escr")
        nc.scalar.activation(escr[:], kf[1]["c1"][0:1, 0:1], AF.Exp)

        # ---------------- main matmuls ----------------
        sc_ps = psc.tile([NQ, KPAD], F32, name="sc", tag="sc")[:, :]
        i = 0
        for (li, aname, kname, m) in PAIRS:
            for hc in range(2):
                nc.tensor.matmul(sc_ps, af[(li, aname, hc)][:],
                                 kf[li][kname][:, hc * KPAD:(hc + 1) * KPAD],
                                 start=(i == 0), stop=False)
                i += 1

        # alpha row: u_q + leaf corrections
        aps = acc[0:1, 0:NQ]
        ai, n_alpha = 0, 6
        for dc in range(2):
            nc.tensor.matmul(aps, u_q_bf[dc], qT[dc][:],
                             start=(ai == 0), stop=(ai == n_alpha - 1)); ai += 1
        for li in range(2):
            for hc in range(2):
                nc.tensor.matmul(aps, neg_m05[:, 0:1],
                                 af[(li, "sp8", hc)][:],
                                 start=(ai == 0), stop=(ai == n_alpha - 1)); ai += 1
        AE = apool.tile([64, NQ], BF, name="AE", tag="AE")
        nc.vector.memset(AE[:], 0.0)
        nc.vector.tensor_copy(AE[0:1, :], aps)
        nc.vector.memset(AE[32:33, :], 1.0)

        # beta row: u_k + leaf corrections + mask
        bps = pbp.tile([1, KPAD], F32, name="bps", tag="bps")[0:1, :]
        bi, n_beta = 0, 6
        for dc in range(2):
            nc.tensor.matmul(bps, u_k_bf[dc],
                             kTb[:, dc * KPAD:(dc + 1) * KPAD],
                             start=(bi == 0), stop=(bi == n_beta - 1)); bi += 1
        for li in range(2):
            for hc in range(2):
                nc.tensor.matmul(bps, u8[li][:, hc:hc + 1],
                                 kf[li]["sp8"][:, hc * KPAD:(hc + 1) * KPAD],
                                 start=(bi == 0), stop=(bi == n_beta - 1)); bi += 1
        brow = apool.tile([1, KPAD], F32, name="brow", tag="brow")
        nc.vector.tensor_tensor(brow[0:1, :], bps, m01[0:1, :], ALU.add)
        BE = apool.tile([64, KPAD], BF, name="BE", tag="BE")
        nc.vector.memset(BE[:], 0.0)
        nc.vector.memset(BE[0:1, :], 1.0)
        nc.vector.tensor_copy(BE[32:33, :], brow[0:1, :])
        nc.tensor.matmul(sc_ps, AE[:, :], BE[:, :], start=False, stop=True)

        # ---------------- exp (no max subtraction) + AV ----------------
        E_t = wpool.tile([NQ, KPAD], BF, name="Et", tag="Et")
        zsb = wpool.tile([NQ, 1], F32, name="zsb", tag="zsb")
        nc.scalar.activation(E_t[:], sc_ps, AF.Exp, accum_out=zsb[:, 0:1])

        v_bf = kpool.tile([128, KC * DV], BF, name="vbf", tag="vbf")
        nc.scalar.copy(v_bf[:], vin[:])
        for jc in range(KC):
            ps = ptp.tile([128, 128], BF, name="tpe", tag="tp")
            nc.tensor.transpose(ps[:], E_t[:, jc * 128:(jc + 1) * 128], ident[:])
            et = wpool.tile([128, NQ], BF, name=f"et{jc % 2}", tag=f"et{jc % 2}")
            nc.vector.tensor_copy(et[:], ps[:])
            nc.tensor.matmul(ov_ps[:], et[:], v_bf[:, jc * 256:(jc + 1) * 256],
                             start=(jc == 0), stop=(jc == KC - 1))
        out_sb = wpool.tile([NQ, DV], F32, name="outsb", tag="outsb")
        nc.vector.tensor_copy(out_sb[:], ov_ps[:])
        nc.sync.dma_start(d_ov[:], out_sb[:])
        nc.sync.dma_start(d_z[:], zsb[:])

    nc.finalize()
    return nc


_CACHE = {}


def _plan(vl):
    """Key-shard plan: per-core KPAD and (batch, key-offset) assignments."""
    chunks = [max(1, (int(v) + 127) // 128) for v in vl]
    L = 1
    while sum((c + L - 1) // L for c in chunks) > 8:
        L += 1
    KPAD = 128 * L
    assign = []
    for b in range(B):
        for i in range((chunks[b] + L - 1) // L):
            assign.append((b, i * KPAD))
    live = len(assign)
    while len(assign) < 8:
        assign.append((0, 0))
    return KPAD, assign, live


def _in_maps(queries, keys, values, vl, Wq, Wk, wv_c, KPAD, assign):
    uq = np.ascontiguousarray((C0 * (Wq.T @ wv_c)).reshape(2, 128).T.astype(np.float32))
    uk = np.ascontiguousarray((C0 * (Wk.T @ wv_c)).reshape(2, 128).T.astype(np.float32))
    maps = []
    for (b, off) in assign:
        end = min(NK, off + KPAD)
        kb = np.zeros((KPAD, H), dtype=np.float32)
        vb = np.zeros((KPAD, DV), dtype=np.float32)
        kb[:end - off] = keys[b, off:end]
        vb[:end - off] = values[b, off:end]
        maps.append({
            "queries": np.ascontiguousarray(queries[b]),
            "keys": kb, "values": vb,
            "Wq": Wq, "Wk": Wk, "wv": wv_c, "uq": uq, "uk": uk,
            "vl": np.array([[float(max(int(vl[b]) - off, 1))]], dtype=np.float32),
        })
    return maps


def _combine(results, assign, live):
    ov = np.zeros((B, NQ, DV), dtype=np.float32)
    z = np.zeros((B, NQ, 1), dtype=np.float32)
    for c in range(live):
        b, _ = assign[c]
        ov[b] += results[c]["ov"]
        z[b] += results[c]["z"]
    return ov / z


def kernel(queries, keys, values, valid_lens, Wq, Wk, wv):
    queries = np.ascontiguousarray(queries, dtype=np.float32)
    keys = np.ascontiguousarray(keys, dtype=np.float32)
    values = np.ascontiguousarray(values, dtype=np.float32)
    Wq = np.ascontiguousarray(Wq, dtype=np.float32)
    Wk = np.ascontiguousarray(Wk, dtype=np.float32)
    wv_c = np.ascontiguousarray(np.asarray(wv).reshape(H, 1), dtype=np.float32)
    vl = np.asarray(valid_lens).astype(np.int64).reshape(B)

    KPAD, assign, live = _plan(vl)
    if KPAD not in _CACHE:
        _CACHE[KPAD] = build_program(KPAD)
    nc = _CACHE[KPAD]

    maps = _in_maps(queries, keys, values, vl, Wq, Wk, wv_c, KPAD, assign)
    res = run_bass_kernel_spmd(nc, maps, list(range(8))).results
    return _combine(res, assign, live)


if __name__ == "__main__":
    d = np.load("/tmp/additive_attn_ref.npz")
    out = kernel(**{k: d[k] for k in
                    ["queries", "keys", "values", "valid_lens", "Wq", "Wk", "wv"]})
    ref = d["out"]
    print("rel err:", np.linalg.norm(out - ref) / np.linalg.norm(ref))
    print("max abs err:", np.abs(out - ref).max())
